# revision 39
# baseline (speedup 1.0000x reference)
# MoE-routing kernel for Trainium2: out[b] = x[b] @ weight[y[b]] + bias[y[b]]
# x: [1024, 64, 1152] f32, y: [1024] int64, weight: [1000, 1152, 128] f32,
# bias: [1000, 128] f32 -> out: [1024, 64, 128] f32.
#
# Strategy: data-parallel over batch with class-dedup. The Bass program is
# built AFTER seeing y: samples are sorted by class and samples sharing a
# class form a chunk (<=8, PSUM bank limit); each chunk loads its class
# weight from HBM once and streams all member samples' x through the PE as
# the moving operand. Chunk-size counts are canonicalized (split/pad) to be
# divisible by 8 so one SPMD program serves all cores. The kernel is
# HBM-bandwidth bound, so precision is spent where it buys accuracy per
# byte: weights travel as fp8 e3m4 (x128 scale, 4 mantissa bits), x as
# e3m4 (x2 scale) for 7 of 9 k-tiles and bf16 (x2) for the rest, out as
# bf16 — ~26 MB per core vs 59 MB for the naive per-sample bf16 gather,
# at a rel-err of 1.78e-2 (deterministic, sim == HW) vs the 2e-2 gate.
# Per (chunk, ktile): stationary = w-ktile [128,128] (FWL-eligible fp8),
# moving = x [128, 64*n], accumulating over 9 ktiles into PSUM [128, n*64];
# output is written transposed ([out_dim, slot, row]) and fixed on host.
# The X bf16/fp8 k-tiles ride in ONE packed DRAM tensor (fp8 tail accessed
# via bitcast) so each supergroup needs only two DMA descriptors; dummy
# matmuls on a zeroed tile keep the PE's HAM clock-gate at 2.4 GHz through
# the DMA ramp and the per-supergroup arrival gaps.

import numpy as np

B, N, HIDDEN = 1024, 64, 1152
NUM_CLASSES = 1000
OUT_DIM = 128
KT = HIDDEN // 128  # 9 k-tiles
NB16 = 2            # k-tiles 0..1 carried in bf16
NF8 = KT - NB16     # k-tiles 3..8 carried in e3m4
NCORES = 8
CHUNK_MAX = 8       # 64*8 f32 = 2KB = one PSUM bank
W_SCALE = 128.0     # w ~N(0,0.02) -> e3m4's [0.25, 15.5] normal range
X_SCALE = 2.0       # x ~N(0,1) -> e3m4 range (max|2x| ~ 10.8 < 15.5)
SG_BUDGETS = [8, 8]  # lead-in supergroup slot budgets (DMA ramp)
SG_SLOTS = 16        # steady-state supergroup slot budget

_cache = {}


def _schedule(y):
    """Data-dependent, core-uniform schedule.

    Returns (sizes_sched, sg_list, chunk_class[8,NCHUNK], slot_sample[8,NSLOT],
    slot_valid[8,NSLOT]). sizes_sched/sg_list are identical for every core so
    a single SPMD program serves all 8."""
    yi = np.asarray(y).astype(np.int64)
    order = np.argsort(yi, kind="stable")
    chunks = []  # (size, class, [sample indices]) ; pad chunks use class 0/-1
    i = 0
    nb = len(yi)
    while i < nb:
        j = i
        while j < nb and yi[order[j]] == yi[order[i]]:
            j += 1
        run = order[i:j]
        for s in range(0, len(run), CHUNK_MAX):
            part = run[s : s + CHUNK_MAX]
            chunks.append((len(part), int(yi[order[i]]), [int(v) for v in part]))
        i = j
    bysize = {}
    for ch in chunks:
        bysize.setdefault(ch[0], []).append(ch)
    # Canonicalize: every size-count divisible by NCORES. Splitting a chunk
    # costs one extra W load (~147KB); padding costs (x+out) bytes per slot
    # (~120KB/slot). Pick the cheaper fix per size level; splits only create
    # strictly smaller sizes, which are processed later.
    for v in range(CHUNK_MAX, 1, -1):
        lst = bysize.get(v, [])
        r = len(lst) % NCORES
        if r == 0:
            continue
        if r * 147.0 <= (NCORES - r) * v * 120.0:
            for _ in range(r):
                sz, c, samps = lst.pop()
                a, b = v // 2, v - v // 2
                bysize.setdefault(a, []).append((a, c, samps[:a]))
                bysize.setdefault(b, []).append((b, c, samps[a:]))
        else:
            for _ in range(NCORES - r):
                lst.append((v, 0, [-1] * v))
    lst1 = bysize.setdefault(1, [])
    r = len(lst1) % NCORES
    if r:
        for _ in range(NCORES - r):
            lst1.append((1, 0, [-1]))
    # Deal each size's chunks round-robin to cores, then emit sizes in a
    # proportional interleave so every supergroup carries the same mix of
    # chunk sizes (uniform per-sg LDWEIGHTS/matmul load on the PE).
    per_core_by_size = {}
    per_core_count = {}
    for v in sorted(bysize.keys(), reverse=True):
        lst = bysize[v]
        if not lst:
            continue
        assert len(lst) % NCORES == 0
        per_core_by_size[v] = [
            [lst[i] for i in range(c, len(lst), NCORES)] for c in range(NCORES)
        ]
        per_core_count[v] = len(lst) // NCORES
    merged = []
    for v, cnt in per_core_count.items():
        merged += [((i + 0.5) / cnt, v, i) for i in range(cnt)]
    merged.sort()
    sizes_sched = [v for _, v, _ in merged]
    core_chunks = [
        [per_core_by_size[v][c][i] for _, v, i in merged] for c in range(NCORES)
    ]
    nchunk = len(sizes_sched)
    nslot = sum(sizes_sched)
    # supergroups: cut before exceeding the slot budget; small lead-in
    # budgets (DMA ramp) and a tapered tail (shortens the serial drain of
    # the final compute+out-DMA chain after the last input byte lands)
    sg_list = []
    bi, c_start, s_start, acc = 0, 0, 0, 0
    for ci, v in enumerate(sizes_sched):
        rem = nslot - (s_start + acc)
        if bi < len(SG_BUDGETS):
            budget = SG_BUDGETS[bi]
        elif rem > 19:
            budget = SG_SLOTS
        elif rem > 11:
            budget = 8
        elif rem > 5:
            budget = 4
        else:
            budget = 2
        if acc > 0 and acc + v > budget:
            sg_list.append((c_start, ci, s_start, s_start + acc))
            c_start, s_start, acc = ci, s_start + acc, 0
            bi += 1
        acc += v
    if acc:
        sg_list.append((c_start, nchunk, s_start, s_start + acc))

    chunk_class = np.zeros((NCORES, nchunk), np.int64)
    slot_sample = np.zeros((NCORES, nslot), np.int64)
    slot_valid = np.zeros((NCORES, nslot), bool)
    for c in range(NCORES):
        off = 0
        for ci, (sz, cls, samps) in enumerate(core_chunks[c]):
            assert sz == sizes_sched[ci]
            chunk_class[c, ci] = cls
            for s in samps:
                if s >= 0:
                    slot_sample[c, off] = s
                    slot_valid[c, off] = True
                off += 1
    return sizes_sched, sg_list, chunk_class, slot_sample, slot_valid


def _build_nc(sizes_sched, sg_list):
    import concourse.bass as bass
    import concourse.mybir as mybir
    from concourse.tile import TileContext

    nc = bass.Bass()
    f32 = mybir.dt.float32
    bf16 = mybir.dt.bfloat16
    f8 = mybir.dt.float8e3
    nchunk = len(sizes_sched)
    nslot = sum(sizes_sched)
    # one X tensor per slot: [bf16 ktiles 0..NB16 | fp8 ktiles NB16..9] packed
    # in 768 bytes, declared bf16; fp8 k-tiles are accessed via bitcast
    xrow = (NB16 * N * 2 + NF8 * N) // 2  # bf16 elements per slot
    Xd = nc.declare_dram_parameter("xin", [128, nslot, xrow], bf16, isOutput=False)
    Wd = nc.declare_dram_parameter("win", [128, nchunk, KT, OUT_DIM], f8, isOutput=False)
    Od = nc.declare_dram_parameter("o", [128, nslot, N], bf16, isOutput=True)

    with TileContext(nc) as tc:
        with (
            tc.tile_pool(name="xp", bufs=5) as xp,
            tc.tile_pool(name="wp", bufs=5) as wp,
            tc.tile_pool(name="op", bufs=5) as op,
            tc.tile_pool(name="pp", bufs=7, space="PSUM") as pp,
            tc.tile_pool(name="zp", bufs=1) as zp,
            tc.tile_pool(name="zpp", bufs=1, space="PSUM") as zpp,
        ):
            # HAM management: the PE clock-gates to 1.2 GHz after ~3.4us of
            # idle. A dummy-matmul burst warms it during the DMA ramp;
            # arrival-triggered dummies (reading freshly-landed tiles) keep
            # it warm through the ramp; the first real matmul is gated on
            # sg1's X tile so the PE starts with a DMA cushion buffered; and
            # per-supergroup pad dummies keep its idle slices below the
            # ~3.4us re-throttle window in steady state.
            zt = zp.tile([128, 512], bf16, tag="zt")
            nc.scalar.memzero(zt)
            zps = zpp.tile([128, 512], f32, tag="zps")

            def dummy(rhs, width=128):
                nc.tensor.matmul(
                    zps[:, :width], zt[:, :128], rhs, start=True, stop=True
                )

            nsg = len(sg_list)
            tiles = {}

            def issue(i):
                c0, c1, s0, s1 = sg_list[i]
                wt = wp.tile([128, c1 - c0, KT, OUT_DIM], f8, tag="wt")
                nc.sync.dma_start(out=wt, in_=Wd[:, c0:c1])
                xt = xp.tile([128, s1 - s0, xrow], bf16, tag="xt")
                nc.sync.dma_start(out=xt, in_=Xd[:, s0:s1])
                tiles[i] = (wt, xt)

            # tiny priming transfer: spins the DMA ring up during the
            # preamble so the first real descriptor starts sooner
            prime = zp.tile([128, 16], f8, tag="pr")
            nc.sync.dma_start(out=prime, in_=Wd[:, 0, 0, :16])
            for _ in range(10):
                dummy(zt, width=512)
            for i in range(min(5, nsg)):
                issue(i)

            cp = 0
            for sgi, (c0, c1, s0, s1) in enumerate(sg_list):
                if sgi + 5 < nsg:
                    issue(sgi + 5)
                m = s1 - s0
                wt, xt = tiles.pop(sgi)
                xt8 = xt.bitcast(f8)  # [128, m, 2*xrow]
                ot = op.tile([128, m, N], bf16, tag="ot")
                off = 0
                for ci in range(c0, c1):
                    n = sizes_sched[ci]
                    ps = pp.tile([128, n, N], f32, tag="ps")
                    for k in range(KT):
                        if k < NB16:
                            rhs = xt[:, off : off + n, k * N : (k + 1) * N]
                        else:
                            b0 = NB16 * N * 2 + (k - NB16) * N
                            rhs = xt8[:, off : off + n, b0 : b0 + N]
                        nc.tensor.matmul(
                            ps,
                            wt[:, ci - c0, k, :],
                            rhs,
                            start=(k == 0),
                            stop=(k == KT - 1),
                        )
                    if cp % 2 == 0:
                        nc.vector.tensor_copy(ot[:, off : off + n, :], ps)
                    else:
                        nc.scalar.copy(ot[:, off : off + n, :], ps)
                    cp += 1
                    off += n
                nc.scalar.dma_start(out=Od[:, s0:s1], in_=ot)
                if sgi < nsg - 5:
                    # pad PE busy-time toward the DMA pace; the sg+2 arrival
                    # dummies fire mid-gap (when those tiles land), slicing
                    # the idle window below the HAM re-throttle threshold
                    for _ in range(6 * m // SG_SLOTS):
                        dummy(zt, width=512)
                    if sgi + 2 in tiles:
                        dummy(tiles[sgi + 2][0][:, 0, 0, :])
                        dummy(tiles[sgi + 2][1].bitcast(f8)[:, 0, :128])

    _split_excess_waits(nc)
    nc.finalize()
    _split_excess_waits(nc)
    return nc


def _split_excess_waits(nc, max_waits=1):
    # walrus codegen rejects instructions with >max sync waits; Tile's tail
    # drain can carry several. Hoist the excess onto preceding no-ops.
    import concourse.mybir as mybir

    for f in nc.m.functions:
        for b in f.blocks:
            i = 0
            while i < len(b.instructions):
                inst = b.instructions[i]
                si = inst.sync_info
                if si is not None and len(si.on_wait) > max_waits:
                    excess = list(si.on_wait[:-max_waits])
                    si.on_wait = list(si.on_wait[-max_waits:])
                    for w in excess:
                        nop = mybir.InstNoOp(
                            name=nc.get_next_instruction_name(),
                            engine=inst.engine,
                            sync_info=mybir.SyncInfo(on_wait=[w], on_update=[]),
                            bass_nofuse=True,
                        )
                        nc.register_instruction(nop)
                        b.instructions.insert(i, nop)
                        i += 1
                i += 1


def _prep_inputs(x, weight, chunk_class, slot_sample):
    import ml_dtypes

    bf16 = ml_dtypes.bfloat16
    e3 = ml_dtypes.float8_e3m4
    x = np.ascontiguousarray(x, dtype=np.float32)
    weight = np.ascontiguousarray(weight, dtype=np.float32)
    # x[b, j, 128k+p] * 2 -> per (p, b): [bf16 ktiles 0..NB16 | e3m4 ktiles
    # NB16..9] packed into 768 bytes, viewed as bf16 for the DMA
    xs = (x * X_SCALE).reshape(B, N, KT, 128)
    Xp16 = np.ascontiguousarray(
        xs[:, :, :NB16].astype(bf16).transpose(3, 0, 2, 1)
    )  # [128, B, NB16, 64]
    Xp8 = np.ascontiguousarray(
        np.clip(xs[:, :, NB16:], -15.5, 15.5).astype(e3).transpose(3, 0, 2, 1)
    )  # [128, B, NF8, 64]
    Xp = np.concatenate(
        [
            Xp16.reshape(128, B, NB16 * N).view(np.uint8),
            Xp8.reshape(128, B, NF8 * N).view(np.uint8),
        ],
        axis=2,
    ).view(bf16)  # [128, B, xrow]
    # weight[c, 128k+p, o] * 128 -> e3m4 -> Wp[p, c, k, o]
    wq = np.clip(weight * W_SCALE, -15.5, 15.5).astype(e3)
    Wp = np.ascontiguousarray(
        wq.reshape(NUM_CLASSES, KT, 128, OUT_DIM).transpose(2, 0, 1, 3)
    )
    Xc = [np.ascontiguousarray(Xp[:, slot_sample[c]]) for c in range(NCORES)]
    Wg = [np.ascontiguousarray(Wp[:, chunk_class[c]]) for c in range(NCORES)]
    return Xc, Wg


def kernel(x, y, weight, bias):
    from concourse.bass_utils import run_bass_kernel_spmd

    yi = np.asarray(y).astype(np.int64)
    key = yi.tobytes()
    if _cache.get("key") != key:
        sizes_sched, sg_list, chunk_class, slot_sample, slot_valid = _schedule(yi)
        _cache.update(
            key=key,
            nc=_build_nc(sizes_sched, sg_list),
            chunk_class=chunk_class,
            slot_sample=slot_sample,
            slot_valid=slot_valid,
        )
    nc = _cache["nc"]
    slot_sample = _cache["slot_sample"]
    slot_valid = _cache["slot_valid"]

    Xc, Wg = _prep_inputs(x, weight, _cache["chunk_class"], slot_sample)
    in_maps = [{"xin": Xc[c], "win": Wg[c]} for c in range(NCORES)]
    res = run_bass_kernel_spmd(nc, in_maps, list(range(NCORES)), **_cache.get("runkw", {}))
    _cache["last_result"] = res

    inv = 1.0 / (W_SCALE * X_SCALE)
    out = np.empty((B, N, OUT_DIM), np.float32)
    for c in range(NCORES):
        oc = np.asarray(res.results[c]["o"], dtype=np.float32) * inv
        valid = slot_valid[c]
        out[slot_sample[c][valid]] = oc[:, valid, :].transpose(1, 2, 0)
    out += np.asarray(bias, dtype=np.float32)[yi][:, None, :]
    return out
